# revision 56
# baseline (speedup 1.0000x reference)
"""Block-causal (block=64) MHA + qkv/out projections on 8 NeuronCores.

Sharding: 8 cores = 2 batches x 4 head-groups (4 heads each).
Per core: qkv projection for its heads, block-causal attention for 4 heads
(processed as 2 head-pairs packed across the 128 partitions), partial output
projection over its 256 channels. Host sums the 4 bf16 partials per batch
and adds the bias.

All matmuls run in bf16 at the full 1-cycle/row PE rate; LDWEIGHTS overlap
the previous matmul's stream, so PE time ~= total moving rows (~114us/core).
The Scalar engine's exp over the causal score volume (~100us) is the other
near-critical engine, so the emission is one global software pipeline:
projection (phase 1/2) and out-projection chunks sit in a filler queue and
are popped between attention steps on a per-step budget (attention's PE
deficit vs the exp time), keeping the PE busy while the Scalar engine works
through the exps; out-proj items are held back for the Scalar-gated last
block and the tail. Phase work for query block qi+1 is enqueued at block qi
and force-drained before block qi+1's attention reads it (emission order
defines the dependency order the Tile framework sees). The prologue emits
only what block 0 pair 0 needs (q/k tiles dt 0/2 + its diag v tiles);
pair 1's projections are fillers inside pair 0's steps, so the exp stream
starts ~5us earlier. Inputs DMA in per-512-column chunks from two issue
queues, first-needed first, so the phase-1 accumulation chain starts on
first arrival; y leaves per 128x512 half-tile, the last ones split across
otherwise-idle issue queues.

On-chip layout is feature-major (transposed): scores are computed transposed
(S^T[k, q] = k . q) so no on-chip transposes are needed anywhere.

Softmax denominators come for free from the PV matmul: each V tile carries a
constant ones column (A tiles at M=64 with values in 0:64; B tiles at M=0
with values in 64:128), so the A accumulator's row 64 and the B accumulator's
row 0 hold the running sum(exp), and B's normalized rows land directly on
attnT partitions 64:128 (no partition-shift DMA). Reciprocals: the two
denominator rows are staged to SBUF, DMA-scattered into a [64,16] tile so the
BITWISE_NOT-seed + 2 Newton-Raphson steps run across 64 partitions instead of
one lane, then DMA'd back to rows and broadcast across partitions with K=1
(-1)-matmuls (NR yields MINUS the reciprocal; the -1 cancels the sign).

PSUM: 2 mm ring banks (projections / out-proj / reciprocal-broadcasts),
1 bank per score half, 2x2 accumulator banks so consecutive pairs never
contend.
"""

from collections import deque

import numpy as np
import ml_dtypes

import concourse.bass as bass
import concourse.tile as tile
from concourse import bacc
from concourse import mybir

B, N, C = 2, 2048, 1024
H, HD = 16, 64
HPC = 4  # heads per core
CSL = HPC * HD  # 256 channel slice per core
QKW = 2 * CSL  # 512: q then k output channels
NCORES = 8
QBLK = 512
NQB = N // QBLK  # 4
NT = N // 128  # 16 seq tiles of 128
SCALE = HD**-0.5
F32 = mybir.dt.float32
BF16 = mybir.dt.bfloat16
I32 = mybir.dt.int32
EXP = mybir.ActivationFunctionType.Exp
COPY = mybir.ActivationFunctionType.Copy
XOR = mybir.AluOpType.bitwise_xor
ADD = mybir.AluOpType.add


def build_nc():
    nc = bacc.Bacc("TRN2", target_bir_lowering=False, debug=False, num_devices=NCORES)

    xT_d = nc.dram_tensor("xT", [8, 128, N], BF16, kind="ExternalInput")
    wqk_d = nc.dram_tensor("wqkT", [8, 128, QKW], BF16, kind="ExternalInput")
    wv_d = nc.dram_tensor("wvT", [8, 128, CSL], BF16, kind="ExternalInput")
    wp_d = nc.dram_tensor("wpT", [2, 128, C], BF16, kind="ExternalInput")
    y_d = nc.dram_tensor("y", [N, C], BF16, kind="ExternalOutput")

    with tile.TileContext(nc) as tc:
        with (
            tc.tile_pool(name="persist", bufs=1) as persist,
            tc.tile_pool(name="pt", bufs=3) as pt_pool,
            tc.tile_pool(name="bs", bufs=2) as bs_pool,
            tc.tile_pool(name="nrm", bufs=2) as nrm_pool,
            tc.tile_pool(name="yout", bufs=3) as y_pool,
            tc.tile_pool(name="psmm", bufs=2, space="PSUM") as ps_mm,
            tc.tile_pool(name="pssc", bufs=1, space="PSUM") as ps_sc,
            tc.tile_pool(name="psacc", bufs=2, space="PSUM") as ps_acc,
        ):
            # ---- persistent tiles ----
            wqk_bf = [persist.tile([128, QKW], BF16, tag=f"wqk{i}", name=f"wqk{i}") for i in range(8)]
            wv_bf = [persist.tile([128, CSL], BF16, tag=f"wv{i}", name=f"wv{i}") for i in range(8)]
            wp_bf = [persist.tile([128, C], BF16, tag=f"wp{i}", name=f"wp{i}") for i in range(2)]
            x_bf = [persist.tile([128, N], BF16, tag=f"xb{i}", name=f"xb{i}") for i in range(8)]
            qkT = [persist.tile([128, N], BF16, tag=f"qk{t}", name=f"qk{t}") for t in range(4)]
            # vA[nt] per pair p at cols 128p: [v(64) | 1 | 0*63]  -> A out on rows 0:64, den row 64
            # vB[nt] per pair p at cols 128p: [1 | 0*63 | v(64)]  -> B out on rows 64:128, den row 0
            vA = [persist.tile([128, 256], BF16, tag=f"vA{t}", name=f"vA{t}") for t in range(NT)]
            vB = [persist.tile([128, 256], BF16, tag=f"vB{t}", name=f"vB{t}") for t in range(NT)]
            attnT = [persist.tile([128, N], BF16, tag=f"at{p}", name=f"at{p}") for p in range(2)]

            # ---- DMA in: interleave wqk/x(nb=0) per ct on two issue engines
            # so phase 1's ct-accumulation chain can start on first arrival
            for ct in range(8):
                nc.scalar.dma_start(out=wqk_bf[ct], in_=wqk_d[ct])
                nc.sync.dma_start(out=x_bf[ct][:, 0:QBLK], in_=xT_d[ct][:, 0:QBLK])
            for ct in range(8):
                nc.scalar.dma_start(out=wv_bf[ct], in_=wv_d[ct])
            for nb in range(1, NQB):
                sl = slice(nb * QBLK, (nb + 1) * QBLK)
                for ct in range(8):
                    nc.sync.dma_start(out=x_bf[ct][:, sl], in_=xT_d[ct][:, sl])
            for pr in range(2):
                nc.scalar.dma_start(out=wp_bf[pr], in_=wp_d[pr])

            # ---- constants ----
            ones_f = persist.tile([128, 64], F32, tag="onesf")
            nc.vector.memset(ones_f, -1.0)
            onesel = persist.tile([128, 64], BF16, tag="sel")
            nc.vector.tensor_copy(out=onesel[64:65, :], in_=ones_f[64:65, :])
            nc.vector.tensor_copy(out=onesel[0:1, :], in_=ones_f[0:1, :])
            # constant halves of the V tiles (memset once; alternate engines)
            for nt in range(NT):
                vA3 = vA[nt].rearrange("p (g c) -> p g c", c=128)
                vB3 = vB[nt].rearrange("p (g c) -> p g c", c=128)
                nc.vector.memset(vA3[:, :, 65:128], 0.0)
                nc.vector.memset(vA3[:, :, 64:65], 1.0)
                nc.vector.memset(vB3[:, :, 1:64], 0.0)
                nc.vector.memset(vB3[:, :, 0:1], 1.0)

            # ---- phase 1 (qk proj, per nb x dt), phase 2 (v proj, per nt) ----
            def emit_ph1(nb, dt, scalar_copy):
                ps = ps_mm.tile([128, QBLK], F32, tag="mm", name="ps_qk")
                for ct in range(8):
                    nc.tensor.matmul(
                        ps,
                        lhsT=wqk_bf[ct][:, dt * 128 : (dt + 1) * 128],
                        rhs=x_bf[ct][:, nb * QBLK : (nb + 1) * QBLK],
                        start=(ct == 0),
                        stop=(ct == 7),
                    )
                dst = qkT[dt][:, nb * QBLK : (nb + 1) * QBLK]
                if scalar_copy:
                    nc.scalar.activation(out=dst, in_=ps, func=COPY)
                else:
                    nc.vector.tensor_copy(out=dst, in_=ps)

            def emit_ph2(nt):
                ps = ps_mm.tile([128, CSL], F32, tag="mm", name="ps_v")
                for ct in range(8):
                    nc.tensor.matmul(
                        ps,
                        lhsT=x_bf[ct][:, nt * 128 : (nt + 1) * 128],
                        rhs=wv_bf[ct],
                        start=(ct == 0),
                        stop=(ct == 7),
                    )
                ps3 = ps.rearrange("p (g c) -> p g c", c=128)  # [128, 2, 128]
                vA3 = vA[nt].rearrange("p (g c) -> p g c", c=128)
                vB3 = vB[nt].rearrange("p (g c) -> p g c", c=128)
                nc.vector.tensor_copy(out=vA3[:, :, 0:64], in_=ps3[:, :, 0:64])
                nc.vector.tensor_copy(out=vB3[:, :, 64:128], in_=ps3[:, :, 64:128])

            # ---- out-projection, one nt tile at a time; y goes out in cb
            # halves (separate casts + DMAs so the last tiles drain fast;
            # the tail variant splits across engines that are idle by then)
            def emit_op(nt, tail=False):
                ysb = y_pool.tile([128, C], BF16, tag="y", name="ysb")
                for cb in range(2):
                    cs = slice(cb * QBLK, (cb + 1) * QBLK)
                    psy = ps_mm.tile([128, QBLK], F32, tag="mm", name="psy")
                    for pr in range(2):
                        nc.tensor.matmul(
                            psy,
                            lhsT=attnT[pr][:, nt * 128 : (nt + 1) * 128],
                            rhs=wp_bf[pr][:, cb * QBLK : (cb + 1) * QBLK],
                            start=(pr == 0),
                            stop=(pr == 1),
                        )
                    if tail and cb == 1:
                        nc.scalar.activation(out=ysb[:, cs], in_=psy, func=COPY)
                        nc.scalar.dma_start(
                            out=y_d[nt * 128 : (nt + 1) * 128, cs], in_=ysb[:, cs]
                        )
                    else:
                        nc.vector.tensor_copy(out=ysb[:, cs], in_=psy)
                        nc.sync.dma_start(
                            out=y_d[nt * 128 : (nt + 1) * 128, cs], in_=ysb[:, cs]
                        )

            # ---- softmax normalization (per pair, per query block) ----
            # split in two: `pre` (den staging + DMA scatter + wide NR +
            # DMA gather, no PE work) runs at the next slot's start; `post`
            # (K=1 broadcasts + muls) runs a few steps later so the PE never
            # waits on the DMA/NR chain.
            def make_norm(pair, qs, at_bA, at_bB, tail=False):
                rcrow = nrm_pool.tile([128, QBLK], BF16, tag="rcr", name="rcr")

                def pre():
                    C0, C1, C2 = 0.23549792, 2.0017324, 2.0
                    # stage the two denominator rows (A at p64, B at p0);
                    # the tail chain uses the by-then-idle Scalar engine
                    stage = nrm_pool.tile([128, QBLK], F32, tag="stg", name="stg")
                    if tail:
                        nc.scalar.activation(out=stage[64:65, :], in_=at_bA[64:65, :], func=COPY)
                        nc.scalar.activation(out=stage[0:1, :], in_=at_bB[0:1, :], func=COPY)
                    else:
                        nc.vector.tensor_copy(out=stage[64:65, :], in_=at_bA[64:65, :])
                        nc.vector.tensor_copy(out=stage[0:1, :], in_=at_bB[0:1, :])
                    # scatter across 64 partitions so NR runs wide
                    denT = nrm_pool.tile([64, 16], F32, tag="dnt", name="dnt")
                    nc.sync.dma_start(out=denT[:, 0:8], in_=stage[64:65, :])
                    nc.sync.dma_start(out=denT[:, 8:16], in_=stage[0:1, :])
                    # z0 = ~bits(d) * (-c0): NR in the negated domain
                    # z_{k+1} = (d*z_k + Ck) * z_k; z2 = -1/d and the -1
                    # broadcast row cancels the sign.
                    w1 = nrm_pool.tile([64, 16], F32, tag="w1", name="w1")
                    w2 = nrm_pool.tile([64, 16], F32, tag="w2", name="w2")
                    w3 = nrm_pool.tile([64, 16], F32, tag="w3", name="w3")
                    rcT = nrm_pool.tile([64, 16], BF16, tag="rcT", name="rcT")
                    nc.vector.tensor_scalar(
                        out=w1.bitcast(I32), in0=denT.bitcast(I32),
                        scalar1=-1, scalar2=None, op0=XOR,
                    )
                    nc.vector.tensor_scalar_mul(w2, w1, C0)  # z0
                    nc.vector.tensor_mul(out=w1, in0=denT, in1=w2)
                    nc.vector.tensor_scalar(
                        out=w3, in0=w1, scalar1=C1, scalar2=None, op0=ADD
                    )
                    nc.vector.tensor_mul(out=w1, in0=w2, in1=w3)  # z1
                    nc.vector.tensor_mul(out=w2, in0=denT, in1=w1)
                    nc.vector.tensor_scalar(
                        out=w3, in0=w2, scalar1=C2, scalar2=None, op0=ADD
                    )
                    nc.vector.tensor_mul(out=rcT, in0=w1, in1=w3)  # z2 = -1/d
                    # back to rows (A at p64, B at p0)
                    nc.sync.dma_start(out=rcrow[64:65, :], in_=rcT[:, 0:8])
                    nc.sync.dma_start(out=rcrow[0:1, :], in_=rcT[:, 8:16])

                def post():
                    bcA = ps_mm.tile([128, QBLK], F32, tag="mm", name="bcA")
                    bcB = ps_mm.tile([128, QBLK], F32, tag="mm", name="bcB")
                    nc.tensor.matmul(
                        bcA[0:64, :], lhsT=onesel[64:65, :],
                        rhs=rcrow[64:65, :], start=True, stop=True,
                    )
                    nc.tensor.matmul(
                        bcB[64:128, :], lhsT=onesel[0:1, :],
                        rhs=rcrow[0:1, :], start=True, stop=True,
                    )
                    # stage through SBUF (tensor ops read at most one PSUM in)
                    bsA = bs_pool.tile([128, QBLK], BF16, tag="bsA", name="bsA")
                    bsB = bs_pool.tile([128, QBLK], BF16, tag="bsB", name="bsB")
                    nc.vector.tensor_copy(out=bsA[0:64, :], in_=bcA[0:64, :])
                    nc.vector.tensor_copy(out=bsB[64:128, :], in_=bcB[64:128, :])
                    nc.vector.tensor_mul(
                        out=attnT[pair][0:64, qs], in0=at_bA[0:64, :], in1=bsA[0:64, :]
                    )
                    nc.vector.tensor_mul(
                        out=attnT[pair][64:128, qs], in0=at_bB[64:128, :],
                        in1=bsB[64:128, :],
                    )
                return pre, post

            # ---- prologue: only what attention block 0 pair 0 needs up
            # front (q/k tiles dt 0 and 2 + diag v tiles); pair 1's dt 1/3
            # become fillers inside pair 0's steps ----
            emit_ph1(0, 0, scalar_copy=True)
            emit_ph1(0, 2, scalar_copy=True)
            for nt in range(4):
                emit_ph2(nt)

            # ---- main pipeline ----
            # filler queue: (kind, qi, est_cost_ns, closure). Pops are rationed
            # by a per-step budget (the PE-work deficit of one attention step
            # vs the Scalar engine's exp time) so late, Scalar-gated blocks
            # still have PE work; op items are held back for block 3 and the
            # tail (OP_RESERVE covers the final norm chain's latency).
            STEP_NS = 420.0
            C_PH1, C_PH2, C_OP = 1700.0, 850.0, 850.0
            OP_RESERVE = 4
            fill_q = deque()
            budget = [0.0]
            norm_q = []  # [(qi, pair, pre_fn, post_fn)]
            post_q = []  # [(qi, pair, post_fn)]

            def pop_fillers(qi):
                while fill_q:
                    kind, q_, cost, fn = fill_q[0]
                    if kind == "op":
                        n_op = sum(1 for it in fill_q if it[0] == "op")
                        if qi < NQB - 1 or n_op <= OP_RESERVE:
                            break
                    if budget[0] < cost:
                        break
                    fill_q.popleft()
                    budget[0] -= cost
                    fn()

            def drain(pred):
                for _ in range(len(fill_q)):
                    item = fill_q.popleft()
                    if pred(item):
                        budget[0] -= item[2]
                        item[3]()
                    else:
                        fill_q.append(item)

            def run_post():
                nqi, npair, fn = post_q.pop(0)
                fn()
                if npair == 1:
                    for nt in range(4 * nqi, 4 * nqi + 4):
                        fill_q.append(
                            ("op", nqi, C_OP,
                             (lambda t, tl: lambda: emit_op(t, tl))(nt, nqi == NQB - 1))
                        )

            for dt_ in (1, 3):
                fill_q.append(
                    ("ph1", 0, C_PH1,
                     (lambda d: lambda: emit_ph1(0, d, False))(dt_))
                )

            for qi in range(NQB):
                if qi > 0:
                    # qk-projection for this block must be emitted before its
                    # attention reads qkT
                    drain(lambda it: it[0] == "ph1" and it[1] == qi)
                for pair in range(2):
                    if pair == 1:
                        # pair 1's q/k tiles (dt 1/3) must be in by now
                        drain(lambda it: it[0] == "ph1" and it[1] == qi)
                    if pair == 0 and qi < NQB - 1:
                        for dt_ in range(4):
                            fill_q.append(
                                ("ph1", qi + 1, C_PH1,
                                 (lambda b, d: lambda: emit_ph1(b, d, False))(qi + 1, dt_))
                            )
                    if pair == 1 and qi < NQB - 1:
                        # next block's diagonal v tiles: enqueued a slot
                        # ahead of their deadline (that block's first diag
                        # step) so the DVE copies have latency slack
                        for nt in range(4 * qi + 7, 4 * qi + 3, -1):
                            fill_q.appendleft(
                                ("ph2", qi + 1, C_PH2,
                                 (lambda t: lambda: emit_ph2(t))(nt))
                            )
                    qt = qkT[pair]
                    kt_t = qkT[2 + pair]
                    qs = slice(qi * QBLK, (qi + 1) * QBLK)
                    vsl = slice(pair * 128, (pair + 1) * 128)

                    at_bA = ps_acc.tile([128, QBLK], F32, tag="atA", name="at_bA")
                    at_bB = ps_acc.tile([128, QBLK], F32, tag="atB", name="at_bB")

                    n_reg = 4 * qi
                    total = n_reg + 4
                    at_A, at_B = [0], [0]

                    def fl(cnt, t=total):
                        i = cnt[0]
                        cnt[0] += 1
                        return dict(start=(i == 0), stop=(i == t - 1))

                    steps = [("rect", kt) for kt in range(n_reg)]
                    steps += [("diag", j) for j in range(4)]
                    st = {}

                    def emit_scores(i, qt=qt, kt_t=kt_t, qs=qs, qi=qi, steps=steps, st=st):
                        kind, idx = steps[i]
                        psA = ps_sc.tile([128, QBLK], F32, tag="sA", name="psA")
                        psB = ps_sc.tile([128, QBLK], F32, tag="sB", name="psB")
                        pA = pt_pool.tile([128, QBLK], BF16, tag="pA", name="pA")
                        pB = pt_pool.tile([128, QBLK], BF16, tag="pB", name="pB")
                        if kind == "rect":
                            ks = slice(idx * 128, (idx + 1) * 128)
                            nc.tensor.matmul(
                                psA, lhsT=kt_t[0:64, ks], rhs=qt[0:64, qs],
                                start=True, stop=True,
                            )
                            nc.scalar.activation(out=pA, in_=psA, func=EXP, scale=SCALE)
                            nc.tensor.matmul(
                                psB, lhsT=kt_t[64:128, ks], rhs=qt[64:128, qs],
                                start=True, stop=True,
                            )
                            nc.scalar.activation(out=pB, in_=psB, func=EXP, scale=SCALE)
                            st[i] = ("rect", idx, 0, pA, pB)
                        else:
                            # diagonal tile: one N-restricted full-dst MM per
                            # half; keys 64:128 additionally need q >= q0+64.
                            # The disallowed corner of p holds junk exps that
                            # the split PV below simply never reads.
                            kt = 4 * qi + idx
                            q0 = 128 * idx
                            ks = slice(kt * 128, (kt + 1) * 128)
                            qsl0 = slice(qi * QBLK + q0, (qi + 1) * QBLK)
                            for ph, ps_s, p_s in ((0, psA, pA), (64, psB, pB)):
                                hd_sl = slice(ph, ph + 64)
                                nc.tensor.matmul(
                                    ps_s[:, q0:QBLK], lhsT=kt_t[hd_sl, ks],
                                    rhs=qt[hd_sl, qsl0], start=True, stop=True,
                                )
                                nc.scalar.activation(
                                    out=p_s[:, q0:QBLK], in_=ps_s[:, q0:QBLK],
                                    func=EXP, scale=SCALE,
                                )
                                nc.gpsimd.memset(p_s[64:128, q0 : q0 + 64], 0.0)
                            st[i] = ("diag", kt, q0, pA, pB)

                    def emit_pv(i, vsl=vsl, st=st, fl=fl, at_A=at_A, at_B=at_B,
                                at_bA=at_bA, at_bB=at_bB):
                        kind, kt, q0, pA, pB = st.pop(i)
                        if kind == "rect":
                            nc.tensor.matmul(
                                at_bA, lhsT=vA[kt][:, vsl], rhs=pA, **fl(at_A)
                            )
                            nc.tensor.matmul(
                                at_bB, lhsT=vB[kt][:, vsl], rhs=pB, **fl(at_B)
                            )
                        else:
                            nc.tensor.matmul(
                                at_bA[:, q0:QBLK], lhsT=vA[kt][:, vsl],
                                rhs=pA[:, q0:QBLK], **fl(at_A)
                            )
                            nc.tensor.matmul(
                                at_bB[:, q0:QBLK], lhsT=vB[kt][:, vsl],
                                rhs=pB[:, q0:QBLK], **fl(at_B)
                            )

                    # scores lead PV by two steps (pt_pool bufs=3): the PV of
                    # step i-1 waits on an exp issued two periods earlier, so
                    # the exp -> corner-memset chain never gates the PE, while
                    # scores only overwrite a psum bank one exp back
                    emit_scores(0)
                    while norm_q:
                        nqi, npair, pre_fn, post_fn = norm_q.pop(0)
                        pre_fn()
                        post_q.append((nqi, npair, post_fn))
                    for i in range(len(steps)):
                        if i + 1 < len(steps):
                            if steps[i + 1][0] == "diag" and steps[i][0] == "rect":
                                # this block's diag v tiles are needed next
                                drain(lambda it: it[0] == "ph2" and it[1] == qi)
                            emit_scores(i + 1)
                        if i == 3 and post_q:
                            run_post()
                        budget[0] += STEP_NS
                        pop_fillers(qi)
                        emit_pv(i)
                    pre_fn, post_fn = make_norm(
                        pair, qs, at_bA, at_bB,
                        tail=(qi == NQB - 1 and pair == 1),
                    )
                    norm_q.append((qi, pair, pre_fn, post_fn))

            # ---- tail: start the last norm chain, drain the op reserve
            # (keeps the PE busy during the chain), then its broadcasts and
            # the final out-projection ----
            while norm_q:
                nqi, npair, pre_fn, post_fn = norm_q.pop(0)
                pre_fn()
                post_q.append((nqi, npair, post_fn))
            while fill_q:
                fill_q.popleft()[3]()
            while post_q:
                run_post()
            while fill_q:
                fill_q.popleft()[3]()

    return nc


def _shard_inputs(x, w_qkv, w_proj):
    bf = ml_dtypes.bfloat16
    x = np.ascontiguousarray(np.asarray(x, dtype=np.float32))
    w_qkv = np.asarray(w_qkv, dtype=np.float32)
    w_proj = np.asarray(w_proj, dtype=np.float32)
    xT = [
        np.ascontiguousarray(x[b].T).astype(bf).reshape(8, 128, N) for b in range(B)
    ]
    in_maps = []
    for c in range(NCORES):
        b, g = divmod(c, 4)
        r0 = 64 * HPC * g  # 256 * g
        wq = w_qkv[r0 : r0 + CSL, :]
        wk = w_qkv[C + r0 : C + r0 + CSL, :]
        wvs = w_qkv[2 * C + r0 : 2 * C + r0 + CSL, :]
        wqkT = np.ascontiguousarray(np.concatenate([wq, wk], axis=0).T).astype(bf)
        wvT = np.ascontiguousarray(wvs.T).astype(bf)
        wpT = np.ascontiguousarray(w_proj[:, r0 : r0 + CSL].T).astype(bf)
        in_maps.append(
            {
                "xT": xT[b],
                "wqkT": wqkT.reshape(8, 128, QKW),
                "wvT": wvT.reshape(8, 128, CSL),
                "wpT": wpT.reshape(2, 128, C),
            }
        )
    return in_maps


def run(x, w_qkv, w_proj, b_proj, trace=False, **spmd_kwargs):
    from concourse.bass_utils import run_bass_kernel_spmd

    in_maps = _shard_inputs(x, w_qkv, w_proj)
    nc = build_nc()
    nc.finalize()
    res = run_bass_kernel_spmd(
        nc, in_maps, core_ids=list(range(NCORES)), trace=trace, **spmd_kwargs
    )
    y = np.zeros((B, N, C), np.float32)
    for c in range(NCORES):
        y[c // 4] += np.asarray(res.results[c]["y"], dtype=np.float32)
    y += np.asarray(b_proj, dtype=np.float32)[None, None, :]
    return y, res


def kernel(x, w_qkv, w_proj, b_proj):
    y, _ = run(x, w_qkv, w_proj, b_proj, trace=False)
    return y
